# revision 13
# baseline (speedup 1.0000x reference)
"""Trainium2 Bass kernel for nn_CINComp_18777597018207.

Math: out[b,o,d] = sum_{i,j} W[o, i*39+j] * infeature[b,i,d] * base[b,j,d] + bias[o]

Dataflow (per core, data-parallel over batch, 128 batch elems/core):
  - Reassociate:  out[o,n] = sum_j base[j,n] * Y[(o,j), n],
                  Y[(o,j), n] = sum_i W'[i,(o,j)] * inf[i,n],   n = (b,d)
  - Stage A (PE): Y^T[n, (o,j)] via matmuls, contraction over i (K=200, two
    k-tiles 128+72), float32r at 512-wide moving chunks (full rate), PSUM.
  - Stage B (DVE): one fused custom op per 2048-elem PSUM segment:
    running cumulative sum of Y^T * base_tiled along the (o,j) stream
    (j-inner, padded to 40). Segments chained via per-partition init scalar.
  - Stage C (DVE): per-o sums = differences of the cumsum sampled at
    j==39 positions; add bias; DMA out.
  - ACT builds the repeated base pattern (j mod 40) once per n-chunk.

Self-contained: hardcodes shapes; registers a custom DVE op at import.
"""

import numpy as np

# ---- problem constants (hardcoded per contract) ----
B, INDIM, BASEDIM, D, OUTDIM = 1024, 200, 39, 32, 200
JP = BASEDIM                 # 39, no padding needed (btile handles any period)
F = OUTDIM * JP              # 7800 (o,j) stream length
NCORES = 8
BLOC = B // NCORES           # 128 batch elems per core
NB = 4                       # batch elems per n-chunk
NCHUNK = NB * D              # 128 partitions per n-chunk
NCHUNKS = BLOC // NB         # 32
NSEG = 4                     # PSUM segments per n-chunk
SEGW = 2048                  # full segment width (4 PSUM banks)
CW = 512                     # matmul moving-dim chunk
NCW = 4                      # chunks per segment
SEG_W = [2048, 2048, 2048, 1656]   # seg 3 ends at F=7800
SEG_O = [0, 2048, 4096, 6144]
SEG_CW = [[512] * 4, [512] * 4, [512] * 4, [512, 512, 512, 120]]
TILE_REPS = 55               # 55*39 = 2145 >= 38 (max joff) + 2048

_CUSTOM_OP = None
_NC_CACHE = None


def _get_custom_op():
    """Register TT_MAC_CUMSUM_ANT: out = s0 + cumsum(in0 * in1) along free."""
    global _CUSTOM_OP
    if _CUSTOM_OP is not None:
        return _CUSTOM_OP
    import concourse.dve_ops as dve_ops_mod
    from concourse.dve_ops import DveOp, OPS
    from concourse.dve_spec import Spec, Src0, Src1, C0, AluOp, scan, lower
    from concourse.dve_uop import DveOpSpec

    name = "TT_MAC_CUMSUM_ANT"

    def ref(in0, in1, c0, c1, c2):
        a = np.asarray(in0, np.float32)
        bb = np.broadcast_to(np.asarray(in1, np.float32), a.shape)
        prod = (a * bb).reshape(a.shape[0], -1)
        cs = np.cumsum(prod, axis=1, dtype=np.float32)
        if isinstance(c0, np.ndarray):
            cs = cs + c0.reshape(-1, 1).astype(np.float32)
        else:
            cs = cs + np.float32(c0)
        return cs.reshape(a.shape)

    spec = Spec(body=scan(AluOp.ADD, Src0 * Src1, init=C0), reference=ref)
    shas = {}
    for ver in ("v3", "v4"):
        shas[ver] = DveOpSpec(
            name=name, opcode=0, uops=lower(spec, ver=ver), rd1_en=True
        ).sha(ver)
    op = DveOp(name, spec, subdim=False, uops_sha=shas)
    if name not in dve_ops_mod._SUB_OPCODE_FOR_NAME:
        OPS.append(op)
        dve_ops_mod.CUSTOM_DVE_SPECS[name] = spec
        dve_ops_mod._SUB_OPCODE_FOR_NAME[name] = (
            dve_ops_mod._CUSTOM_DVE_ROW_BASE + len(OPS) - 1
        )
        assert dve_ops_mod._SUB_OPCODE_FOR_NAME[name] < 0x20
    _CUSTOM_OP = op
    return op


def build_nc(reps=1):
    """Build (once) the per-core Bass program. SPMD: same program, 8 cores.

    reps>1 wraps the compute body in a repeat loop (benchmark builds only).
    """
    global _NC_CACHE
    if _NC_CACHE is not None and reps == 1:
        return _NC_CACHE
    import concourse.bacc as bacc
    import concourse.mybir as mybir
    from concourse.tile import TileContext

    op = _get_custom_op()
    f32 = mybir.dt.float32
    f32r = mybir.dt.float32r

    nc = bacc.Bacc("TRN2", debug=False, num_devices=NCORES)
    # inf: [BLOC, INDIM, D] fp32 bits; base2: [BLOC, D, JP]; w: [INDIM, FPAD]
    inf_d = nc.dram_tensor("inf", [INDIM, BLOC * D], f32r, kind="ExternalInput")
    base_d = nc.dram_tensor("basep", [BLOC, D, JP], f32, kind="ExternalInput")
    w_d = nc.dram_tensor("w", [INDIM, F], f32r, kind="ExternalInput")
    # out in [BLOC, D, OUTDIM] layout; host transposes to [BLOC, OUTDIM, D]
    out_d = nc.dram_tensor("out", [BLOC, D, OUTDIM], f32, kind="ExternalOutput")

    with TileContext(nc) as tc:
        with (
            tc.tile_pool(name="wpool", bufs=1) as wpool,
            tc.tile_pool(name="ipool", bufs=1) as ipool,
            tc.tile_pool(name="bse", bufs=3) as bsep,
            tc.tile_pool(name="btl", bufs=2) as btlp,
            tc.tile_pool(name="tmp", bufs=2) as tmpp,
            tc.tile_pool(name="outp", bufs=3) as outp,
            tc.tile_pool(name="ps", bufs=2, space="PSUM") as psp,
        ):
            # whole-core infeature, resident: [i, n] n=(b,d), two k-tiles
            # (loaded first: the first matmul needs inf + W segment 0 only)
            inf0 = ipool.tile([128, BLOC * D], f32r, tag="inf0")
            nc.sync.dma_start(out=inf0[:, :], in_=inf_d[0:128, :])
            inf1 = ipool.tile([72, BLOC * D], f32r, tag="inf1")
            nc.sync.dma_start(out=inf1[:, :], in_=inf_d[128:INDIM, :])
            # persistent weights: W'[i, (o,j)] split into two k-tiles,
            # four segment tiles each (first matmul starts after seg 0 lands)
            w0s, w1s = [], []
            for seg in range(NSEG):
                fo, sw = SEG_O[seg], SEG_W[seg]
                w0 = wpool.tile([128, sw], f32r, tag=f"w0{seg}")
                nc.sync.dma_start(out=w0[:, :], in_=w_d[0:128, fo : fo + sw])
                w0s.append(w0)
                w1 = wpool.tile([72, sw], f32r, tag=f"w1{seg}")
                nc.sync.dma_start(out=w1[:, :], in_=w_d[128:INDIM, fo : fo + sw])
                w1s.append(w1)

            import contextlib

            if reps > 1:
                rep_ctx = tc.For_i(
                    0,
                    reps,
                    1,
                    hint_engines=(
                        mybir.EngineType.PE,
                        mybir.EngineType.DVE,
                        mybir.EngineType.SP,
                    ),
                )
            else:
                rep_ctx = contextlib.nullcontext()
            with rep_ctx:
                for t in range(NCHUNKS):
                    b0 = t * NB
                    n0 = t * NCHUNK
                    lhs0f = inf0[:, n0 : n0 + NCHUNK]
                    lhs1f = inf1[:, n0 : n0 + NCHUNK]

                    # base chunk: [n, j] with n=(b,d) on partitions
                    bch = bsep.tile([128, JP], f32, tag="bch")
                    nc.sync.dma_start(
                        out=bch[:, :],
                        in_=base_d[b0 : b0 + NB, :, :].rearrange(
                            "b d j -> (b d) j"
                        ),
                    )
                    # repeated base pattern along the (o,j) stream (ACT)
                    btile = btlp.tile([128, TILE_REPS * JP], f32, tag="btile")
                    nc.scalar.copy(
                        out=btile[:, :].rearrange("p (r j) -> p r j", j=JP),
                        in_=bch[:, :]
                        .unsqueeze(1)
                        .broadcast_to([128, TILE_REPS, JP]),
                    )

                    tmp = tmpp.tile([128, F], f32, tag="tmp")
                    for seg in range(NSEG):
                        fo, sw = SEG_O[seg], SEG_W[seg]
                        ypsum = psp.tile([128, NCW, CW], f32, tag="ypsum")
                        for ki, (lhsf, wt) in enumerate(
                            ((lhs0f, w0s[seg]), (lhs1f, w1s[seg]))
                        ):
                            co = 0
                            for cw in SEG_CW[seg]:
                                nc.tensor.matmul(
                                    ypsum[:, co // CW, 0:cw],
                                    lhsT=lhsf,
                                    rhs=wt[:, co : co + cw],
                                    start=(ki == 0),
                                    stop=(ki == 1),
                                )
                                co += cw
                        joff = fo % JP
                        init = 0.0 if seg == 0 else tmp[:, fo - 1 : fo]
                        nc.vector._custom_dve(
                            op,
                            out=tmp[:, fo : fo + sw],
                            in0=ypsum[:, :, :].rearrange("p a b -> p (a b)")[
                                :, 0:sw
                            ],
                            in1=btile[:, joff : joff + sw],
                            s0=init,
                        )

                    # compact the cumsum samples at j==JP-1 (one DVE copy),
                    # DMA out; host computes the per-o differences + bias.
                    last = tmp[:, :].rearrange("p (o j) -> p o j", j=JP)[
                        :, :, JP - 1
                    ]
                    nc.sync.dma_start(
                        out=out_d[b0 : b0 + NB, :, :].rearrange(
                            "b d o -> (b d) o"
                        ),
                        in_=last,
                    )
    nc.compile()
    if reps == 1:
        _NC_CACHE = nc
    return nc


def _prep_inputs(infeature, base, W, b):
    """Host-side reshape/pad; returns per-core input maps."""
    infeature = np.asarray(infeature, dtype=np.float32)
    # [B, I, D] -> per-core [I, BLOC*D] contiguous for full-rate DMA
    infT = np.ascontiguousarray(
        infeature.reshape(NCORES, BLOC, INDIM, D).transpose(0, 2, 1, 3)
    ).reshape(NCORES, INDIM, BLOC * D)
    base = np.asarray(base, dtype=np.float32)
    W = np.asarray(W, dtype=np.float32)
    b = np.asarray(b, dtype=np.float32)

    # W'[i, o*39+j] = W[o, i*39+j]
    Wr = W.reshape(OUTDIM, INDIM, BASEDIM)
    Wp = np.ascontiguousarray(
        Wr.transpose(1, 0, 2).reshape(INDIM, F)
    )

    # base2: [B, D, J] (d-major) for single-DMA chunk loads
    base2 = np.ascontiguousarray(base.transpose(0, 2, 1))

    in_maps = []
    for c in range(NCORES):
        s = slice(c * BLOC, (c + 1) * BLOC)
        in_maps.append(
            {
                "inf": infT[c],
                "basep": base2[s],
                "w": Wp,
            }
        )
    return in_maps


def kernel(infeature, base, W, b):
    from concourse.bass_utils import run_bass_kernel_spmd

    nc = build_nc()
    in_maps = _prep_inputs(infeature, base, W, b)
    res = run_bass_kernel_spmd(nc, in_maps, core_ids=list(range(NCORES)))
    # gather cumsum samples [B, D, O]; finish with per-o diff + bias on host
    cum = np.concatenate([res.results[c]["out"] for c in range(NCORES)], axis=0)
    cum = cum.transpose(0, 2, 1)  # [B, O, D]
    out = np.empty_like(cum)
    out[:, 0, :] = cum[:, 0, :]
    np.subtract(cum[:, 1:, :], cum[:, :-1, :], out=out[:, 1:, :])
    out += np.asarray(b, np.float32).reshape(1, OUTDIM, 1)
    return np.ascontiguousarray(out).astype(np.float32)


# revision 14
# speedup vs baseline: 19.9717x; 19.9717x over previous
"""Trainium2 Bass kernel for nn_CINComp_18777597018207.

Math: out[b,o,d] = sum_{i,j} W[o, i*39+j] * infeature[b,i,d] * base[b,j,d] + bias[o]

Dataflow (per core, data-parallel over batch, 128 batch elems/core):
  - Reassociate:  out[o,n] = sum_j base[j,n] * Y[(o,j), n],
                  Y[(o,j), n] = sum_i W'[i,(o,j)] * inf[i,n],   n = (b,d)
  - Stage A (PE): Y^T[n, (o,j)] via matmuls, contraction over i (K=200, two
    k-tiles 128+72), float32r at 512-wide moving chunks (full rate), PSUM.
  - Stage B (DVE): one fused custom op per 2048-elem PSUM segment:
    running cumulative sum of Y^T * base_tiled along the (o,j) stream
    (j-inner, padded to 40). Segments chained via per-partition init scalar.
  - Stage C (DVE): per-o sums = differences of the cumsum sampled at
    j==39 positions; add bias; DMA out.
  - ACT builds the repeated base pattern (j mod 40) once per n-chunk.

Self-contained: hardcodes shapes; registers a custom DVE op at import.
"""

import numpy as np

# ---- problem constants (hardcoded per contract) ----
B, INDIM, BASEDIM, D, OUTDIM = 1024, 200, 39, 32, 200
JP = BASEDIM                 # 39, no padding needed (btile handles any period)
F = OUTDIM * JP              # 7800 (o,j) stream length
NCORES = 8
BLOC = B // NCORES           # 128 batch elems per core
NB = 4                       # batch elems per n-chunk
NCHUNK = NB * D              # 128 partitions per n-chunk
NCHUNKS = BLOC // NB         # 32
NSEG = 4                     # PSUM segments per n-chunk
SEGW = 2048                  # full segment width (4 PSUM banks)
CW = 512                     # matmul moving-dim chunk
NCW = 4                      # chunks per segment
SEG_W = [2048, 2048, 2048, 1656]   # seg 3 ends at F=7800
SEG_O = [0, 2048, 4096, 6144]
SEG_CW = [[512] * 4, [512] * 4, [512] * 4, [512, 512, 512, 120]]
TILE_REPS = 55               # 55*39 = 2145 >= 38 (max joff) + 2048

_CUSTOM_OP = None
_NC_CACHE = None


def _get_custom_op():
    """Register TT_MAC_CUMSUM_ANT: out = s0 + cumsum(in0 * in1) along free."""
    global _CUSTOM_OP
    if _CUSTOM_OP is not None:
        return _CUSTOM_OP
    import concourse.dve_ops as dve_ops_mod
    from concourse.dve_ops import DveOp, OPS
    from concourse.dve_spec import Spec, Src0, Src1, C0, AluOp, scan, lower
    from concourse.dve_uop import DveOpSpec

    name = "TT_MAC_CUMSUM_ANT"

    def ref(in0, in1, c0, c1, c2):
        a = np.asarray(in0, np.float32)
        bb = np.broadcast_to(np.asarray(in1, np.float32), a.shape)
        prod = (a * bb).reshape(a.shape[0], -1)
        cs = np.cumsum(prod, axis=1, dtype=np.float32)
        if isinstance(c0, np.ndarray):
            cs = cs + c0.reshape(-1, 1).astype(np.float32)
        else:
            cs = cs + np.float32(c0)
        return cs.reshape(a.shape)

    spec = Spec(body=scan(AluOp.ADD, Src0 * Src1, init=C0), reference=ref)
    shas = {}
    for ver in ("v3", "v4"):
        shas[ver] = DveOpSpec(
            name=name, opcode=0, uops=lower(spec, ver=ver), rd1_en=True
        ).sha(ver)
    op = DveOp(name, spec, subdim=False, uops_sha=shas)
    if name not in dve_ops_mod._SUB_OPCODE_FOR_NAME:
        OPS.append(op)
        dve_ops_mod.CUSTOM_DVE_SPECS[name] = spec
        dve_ops_mod._SUB_OPCODE_FOR_NAME[name] = (
            dve_ops_mod._CUSTOM_DVE_ROW_BASE + len(OPS) - 1
        )
        assert dve_ops_mod._SUB_OPCODE_FOR_NAME[name] < 0x20
    _CUSTOM_OP = op
    return op


def build_nc(reps=1):
    """Build (once) the per-core Bass program. SPMD: same program, 8 cores.

    reps>1 wraps the compute body in a repeat loop (benchmark builds only).
    """
    global _NC_CACHE
    if _NC_CACHE is not None and reps == 1:
        return _NC_CACHE
    import concourse.bacc as bacc
    import concourse.mybir as mybir
    from concourse.tile import TileContext

    op = _get_custom_op()
    f32 = mybir.dt.float32
    f32r = mybir.dt.float32r

    nc = bacc.Bacc("TRN2", debug=False, num_devices=NCORES)
    # inf: [BLOC, INDIM, D] fp32 bits; base2: [BLOC, D, JP]; w: [INDIM, FPAD]
    inf_d = nc.dram_tensor("inf", [INDIM, BLOC * D], f32r, kind="ExternalInput")
    base_d = nc.dram_tensor("basep", [BLOC, D, JP], f32, kind="ExternalInput")
    w_d = nc.dram_tensor("w", [INDIM, F], f32r, kind="ExternalInput")
    # out in [BLOC, D, OUTDIM] layout; host transposes to [BLOC, OUTDIM, D]
    out_d = nc.dram_tensor("out", [BLOC, D, OUTDIM], f32, kind="ExternalOutput")

    with TileContext(nc) as tc:
        with (
            tc.tile_pool(name="wpool", bufs=1) as wpool,
            tc.tile_pool(name="ipool", bufs=1) as ipool,
            tc.tile_pool(name="bse", bufs=3) as bsep,
            tc.tile_pool(name="btl", bufs=2) as btlp,
            tc.tile_pool(name="tmp", bufs=2) as tmpp,
            tc.tile_pool(name="outp", bufs=3) as outp,
            tc.tile_pool(name="ps", bufs=2, space="PSUM") as psp,
        ):
            # whole-core infeature, resident: [i, n] n=(b,d), two k-tiles
            # (loaded first: the first matmul needs inf + W segment 0 only)
            inf0 = ipool.tile([128, BLOC * D], f32r, tag="inf0")
            nc.sync.dma_start(out=inf0[:, :], in_=inf_d[0:128, :])
            inf1 = ipool.tile([72, BLOC * D], f32r, tag="inf1")
            nc.sync.dma_start(out=inf1[:, :], in_=inf_d[128:INDIM, :])
            # persistent weights: W'[i, (o,j)] split into two k-tiles,
            # four segment tiles each (first matmul starts after seg 0 lands)
            w0s, w1s = [], []
            for seg in range(NSEG):
                fo, sw = SEG_O[seg], SEG_W[seg]
                w0 = wpool.tile([128, sw], f32r, tag=f"w0{seg}")
                nc.sync.dma_start(out=w0[:, :], in_=w_d[0:128, fo : fo + sw])
                w0s.append(w0)
                w1 = wpool.tile([72, sw], f32r, tag=f"w1{seg}")
                nc.sync.dma_start(out=w1[:, :], in_=w_d[128:INDIM, fo : fo + sw])
                w1s.append(w1)

            import contextlib

            if reps > 1:
                rep_ctx = tc.For_i(
                    0,
                    reps,
                    1,
                    hint_engines=(
                        mybir.EngineType.PE,
                        mybir.EngineType.DVE,
                        mybir.EngineType.SP,
                    ),
                )
            else:
                rep_ctx = contextlib.nullcontext()
            with rep_ctx:
                for t in range(NCHUNKS):
                    b0 = t * NB
                    n0 = t * NCHUNK
                    lhs0f = inf0[:, n0 : n0 + NCHUNK]
                    lhs1f = inf1[:, n0 : n0 + NCHUNK]

                    # base chunk: [n, j] with n=(b,d) on partitions
                    bch = bsep.tile([128, JP], f32, tag="bch")
                    nc.sync.dma_start(
                        out=bch[:, :],
                        in_=base_d[b0 : b0 + NB, :, :].rearrange(
                            "b d j -> (b d) j"
                        ),
                    )
                    # repeated base pattern along the (o,j) stream (ACT)
                    btile = btlp.tile([128, TILE_REPS * JP], f32, tag="btile")
                    nc.scalar.copy(
                        out=btile[:, :].rearrange("p (r j) -> p r j", j=JP),
                        in_=bch[:, :]
                        .unsqueeze(1)
                        .broadcast_to([128, TILE_REPS, JP]),
                    )

                    tmp = tmpp.tile([128, F], f32, tag="tmp")
                    for seg in range(NSEG):
                        fo, sw = SEG_O[seg], SEG_W[seg]
                        ypsum = psp.tile([128, NCW, CW], f32, tag="ypsum")
                        for ki, (lhsf, wt) in enumerate(
                            ((lhs0f, w0s[seg]), (lhs1f, w1s[seg]))
                        ):
                            co = 0
                            for cw in SEG_CW[seg]:
                                nc.tensor.matmul(
                                    ypsum[:, co // CW, 0:cw],
                                    lhsT=lhsf,
                                    rhs=wt[:, co : co + cw],
                                    start=(ki == 0),
                                    stop=(ki == 1),
                                )
                                co += cw
                        joff = fo % JP
                        init = 0.0 if seg == 0 else tmp[:, fo - 1 : fo]
                        nc.vector._custom_dve(
                            op,
                            out=tmp[:, fo : fo + sw],
                            in0=ypsum[:, :, :].rearrange("p a b -> p (a b)")[
                                :, 0:sw
                            ],
                            in1=btile[:, joff : joff + sw],
                            s0=init,
                        )

                    # compact the cumsum samples at j==JP-1 (one DVE copy),
                    # DMA out; host computes the per-o differences + bias.
                    last = tmp[:, :].rearrange("p (o j) -> p o j", j=JP)[
                        :, :, JP - 1
                    ]
                    outT = outp.tile([128, OUTDIM], f32, tag="outT")
                    nc.vector.tensor_copy(outT[:, :], last)
                    nc.sync.dma_start(
                        out=out_d[b0 : b0 + NB, :, :].rearrange(
                            "b d o -> (b d) o"
                        ),
                        in_=outT[:, :],
                    )
    nc.compile()
    if reps == 1:
        _NC_CACHE = nc
    return nc


def _prep_inputs(infeature, base, W, b):
    """Host-side reshape/pad; returns per-core input maps."""
    infeature = np.asarray(infeature, dtype=np.float32)
    # [B, I, D] -> per-core [I, BLOC*D] contiguous for full-rate DMA
    infT = np.ascontiguousarray(
        infeature.reshape(NCORES, BLOC, INDIM, D).transpose(0, 2, 1, 3)
    ).reshape(NCORES, INDIM, BLOC * D)
    base = np.asarray(base, dtype=np.float32)
    W = np.asarray(W, dtype=np.float32)
    b = np.asarray(b, dtype=np.float32)

    # W'[i, o*39+j] = W[o, i*39+j]
    Wr = W.reshape(OUTDIM, INDIM, BASEDIM)
    Wp = np.ascontiguousarray(
        Wr.transpose(1, 0, 2).reshape(INDIM, F)
    )

    # base2: [B, D, J] (d-major) for single-DMA chunk loads
    base2 = np.ascontiguousarray(base.transpose(0, 2, 1))

    in_maps = []
    for c in range(NCORES):
        s = slice(c * BLOC, (c + 1) * BLOC)
        in_maps.append(
            {
                "inf": infT[c],
                "basep": base2[s],
                "w": Wp,
            }
        )
    return in_maps


def kernel(infeature, base, W, b):
    from concourse.bass_utils import run_bass_kernel_spmd

    nc = build_nc()
    in_maps = _prep_inputs(infeature, base, W, b)
    res = run_bass_kernel_spmd(nc, in_maps, core_ids=list(range(NCORES)))
    # gather cumsum samples [B, D, O]; finish with per-o diff + bias on host
    cum = np.concatenate([res.results[c]["out"] for c in range(NCORES)], axis=0)
    cum = cum.transpose(0, 2, 1)  # [B, O, D]
    out = np.empty_like(cum)
    out[:, 0, :] = cum[:, 0, :]
    np.subtract(cum[:, 1:, :], cum[:, :-1, :], out=out[:, 1:, :])
    out += np.asarray(b, np.float32).reshape(1, OUTDIM, 1)
    return np.ascontiguousarray(out).astype(np.float32)


# revision 15
# speedup vs baseline: 20.0162x; 1.0022x over previous
"""Trainium2 Bass kernel for nn_CINComp_18777597018207.

Math: out[b,o,d] = sum_{i,j} W[o, i*39+j] * infeature[b,i,d] * base[b,j,d] + bias[o]

Dataflow (per core, data-parallel over batch, 128 batch elems/core):
  - Reassociate:  out[o,n] = sum_j base[j,n] * Y[(o,j), n],
                  Y[(o,j), n] = sum_i W'[i,(o,j)] * inf[i,n],   n = (b,d)
  - Stage A (PE): Y^T[n, (o,j)] via matmuls, contraction over i (K=200, two
    k-tiles 128+72), float32r at 512-wide moving chunks (full rate), PSUM.
  - Stage B (DVE): one fused custom op per 2048-elem PSUM segment:
    running cumulative sum of Y^T * base_tiled along the (o,j) stream
    (j-inner, padded to 40). Segments chained via per-partition init scalar.
  - Stage C (DVE): per-o sums = differences of the cumsum sampled at
    j==39 positions; add bias; DMA out.
  - ACT builds the repeated base pattern (j mod 40) once per n-chunk.

Self-contained: hardcodes shapes; registers a custom DVE op at import.
"""

import numpy as np

# ---- problem constants (hardcoded per contract) ----
B, INDIM, BASEDIM, D, OUTDIM = 1024, 200, 39, 32, 200
JP = BASEDIM                 # 39, no padding needed (btile handles any period)
F = OUTDIM * JP              # 7800 (o,j) stream length
NCORES = 8
BLOC = B // NCORES           # 128 batch elems per core
NB = 4                       # batch elems per n-chunk
NCHUNK = NB * D              # 128 partitions per n-chunk
NCHUNKS = BLOC // NB         # 32
NSEG = 4                     # PSUM segments per n-chunk
SEGW = 2048                  # full segment width (4 PSUM banks)
CW = 512                     # matmul moving-dim chunk
NCW = 4                      # chunks per segment
SEG_W = [2048, 2048, 2048, 1656]   # seg 3 ends at F=7800
SEG_O = [0, 2048, 4096, 6144]
SEG_CW = [[512] * 4, [512] * 4, [512] * 4, [512, 512, 512, 120]]
TILE_REPS = 55               # 55*39 = 2145 >= 38 (max joff) + 2048

_CUSTOM_OP = None
_NC_CACHE = None


def _get_custom_op():
    """Register TT_MAC_CUMSUM_ANT: out = s0 + cumsum(in0 * in1) along free."""
    global _CUSTOM_OP
    if _CUSTOM_OP is not None:
        return _CUSTOM_OP
    import concourse.dve_ops as dve_ops_mod
    from concourse.dve_ops import DveOp, OPS
    from concourse.dve_spec import Spec, Src0, Src1, C0, AluOp, scan, lower
    from concourse.dve_uop import DveOpSpec

    name = "TT_MAC_CUMSUM_ANT"

    def ref(in0, in1, c0, c1, c2):
        a = np.asarray(in0, np.float32)
        bb = np.broadcast_to(np.asarray(in1, np.float32), a.shape)
        prod = (a * bb).reshape(a.shape[0], -1)
        cs = np.cumsum(prod, axis=1, dtype=np.float32)
        if isinstance(c0, np.ndarray):
            cs = cs + c0.reshape(-1, 1).astype(np.float32)
        else:
            cs = cs + np.float32(c0)
        return cs.reshape(a.shape)

    spec = Spec(body=scan(AluOp.ADD, Src0 * Src1, init=C0), reference=ref)
    shas = {}
    for ver in ("v3", "v4"):
        shas[ver] = DveOpSpec(
            name=name, opcode=0, uops=lower(spec, ver=ver), rd1_en=True
        ).sha(ver)
    op = DveOp(name, spec, subdim=False, uops_sha=shas)
    if name not in dve_ops_mod._SUB_OPCODE_FOR_NAME:
        OPS.append(op)
        dve_ops_mod.CUSTOM_DVE_SPECS[name] = spec
        dve_ops_mod._SUB_OPCODE_FOR_NAME[name] = (
            dve_ops_mod._CUSTOM_DVE_ROW_BASE + len(OPS) - 1
        )
        assert dve_ops_mod._SUB_OPCODE_FOR_NAME[name] < 0x20
    _CUSTOM_OP = op
    return op


def build_nc(reps=1):
    """Build (once) the per-core Bass program. SPMD: same program, 8 cores.

    reps>1 wraps the compute body in a repeat loop (benchmark builds only).
    """
    global _NC_CACHE
    if _NC_CACHE is not None and reps == 1:
        return _NC_CACHE
    import concourse.bacc as bacc
    import concourse.mybir as mybir
    from concourse.tile import TileContext

    op = _get_custom_op()
    f32 = mybir.dt.float32
    f32r = mybir.dt.float32r

    nc = bacc.Bacc("TRN2", debug=False, num_devices=NCORES)
    # inf: [BLOC, INDIM, D] fp32 bits; base2: [BLOC, D, JP]; w: [INDIM, FPAD]
    inf_d = nc.dram_tensor("inf", [INDIM, BLOC * D], f32r, kind="ExternalInput")
    base_d = nc.dram_tensor("basep", [BLOC, D, JP], f32, kind="ExternalInput")
    w_d = nc.dram_tensor("w", [INDIM, F], f32r, kind="ExternalInput")
    # out in [BLOC, D, OUTDIM] layout; host transposes to [BLOC, OUTDIM, D]
    out_d = nc.dram_tensor("out", [BLOC, D, OUTDIM], f32, kind="ExternalOutput")

    with TileContext(nc) as tc:
        with (
            tc.tile_pool(name="wpool", bufs=1) as wpool,
            tc.tile_pool(name="ipool", bufs=1) as ipool,
            tc.tile_pool(name="bse", bufs=3) as bsep,
            tc.tile_pool(name="btl", bufs=2) as btlp,
            tc.tile_pool(name="tmp", bufs=2) as tmpp,
            tc.tile_pool(name="outp", bufs=3) as outp,
            tc.tile_pool(name="ps", bufs=2, space="PSUM") as psp,
        ):
            # whole-core infeature, resident: [i, n] n=(b,d), two k-tiles
            # (loaded first: the first matmul needs inf + W segment 0 only)
            inf0 = ipool.tile([128, BLOC * D], f32r, tag="inf0")
            nc.sync.dma_start(out=inf0[:, :], in_=inf_d[0:128, :])
            inf1 = ipool.tile([72, BLOC * D], f32r, tag="inf1")
            nc.sync.dma_start(out=inf1[:, :], in_=inf_d[128:INDIM, :])
            # persistent weights: W'[i, (o,j)] split into two k-tiles,
            # four segment tiles each (first matmul starts after seg 0 lands)
            w0s, w1s = [], []
            for seg in range(NSEG):
                fo, sw = SEG_O[seg], SEG_W[seg]
                w0 = wpool.tile([128, sw], f32r, tag=f"w0{seg}")
                nc.sync.dma_start(out=w0[:, :], in_=w_d[0:128, fo : fo + sw])
                w0s.append(w0)
                w1 = wpool.tile([72, sw], f32r, tag=f"w1{seg}")
                nc.sync.dma_start(out=w1[:, :], in_=w_d[128:INDIM, fo : fo + sw])
                w1s.append(w1)

            import contextlib

            if reps > 1:
                rep_ctx = tc.For_i(
                    0,
                    reps,
                    1,
                    hint_engines=(
                        mybir.EngineType.PE,
                        mybir.EngineType.DVE,
                        mybir.EngineType.SP,
                    ),
                )
            else:
                rep_ctx = contextlib.nullcontext()
            with rep_ctx:
                for t in range(NCHUNKS):
                    b0 = t * NB
                    n0 = t * NCHUNK
                    lhs0f = inf0[:, n0 : n0 + NCHUNK]
                    lhs1f = inf1[:, n0 : n0 + NCHUNK]

                    # base chunk: [n, j] with n=(b,d) on partitions
                    bch = bsep.tile([128, JP], f32, tag="bch")
                    nc.sync.dma_start(
                        out=bch[:, :],
                        in_=base_d[b0 : b0 + NB, :, :].rearrange(
                            "b d j -> (b d) j"
                        ),
                    )
                    # repeated base pattern along the (o,j) stream (ACT)
                    btile = btlp.tile([128, TILE_REPS * JP], f32, tag="btile")
                    nc.scalar.copy(
                        out=btile[:, :].rearrange("p (r j) -> p r j", j=JP),
                        in_=bch[:, :]
                        .unsqueeze(1)
                        .broadcast_to([128, TILE_REPS, JP]),
                    )

                    tmp = tmpp.tile([128, F], f32, tag="tmp")
                    for seg in range(NSEG):
                        fo, sw = SEG_O[seg], SEG_W[seg]
                        ypsum = psp.tile([128, NCW, CW], f32, tag="ypsum")
                        for ki, (lhsf, wt) in enumerate(
                            ((lhs0f, w0s[seg]), (lhs1f, w1s[seg]))
                        ):
                            co = 0
                            for cw in SEG_CW[seg]:
                                nc.tensor.matmul(
                                    ypsum[:, co // CW, 0:cw],
                                    lhsT=lhsf,
                                    rhs=wt[:, co : co + cw],
                                    start=(ki == 0),
                                    stop=(ki == 1),
                                )
                                co += cw
                        joff = fo % JP
                        init = 0.0 if seg == 0 else tmp[:, fo - 1 : fo]
                        nc.vector._custom_dve(
                            op,
                            out=tmp[:, fo : fo + sw],
                            in0=ypsum[:, :, :].rearrange("p a b -> p (a b)")[
                                :, 0:sw
                            ],
                            in1=btile[:, joff : joff + sw],
                            s0=init,
                        )

                    # compact the cumsum samples at j==JP-1 (one DVE copy),
                    # DMA out; host computes the per-o differences + bias.
                    last = tmp[:, :].rearrange("p (o j) -> p o j", j=JP)[
                        :, :, JP - 1
                    ]
                    outT = outp.tile([128, OUTDIM], f32, tag="outT")
                    nc.scalar.copy(outT[:, :], last)
                    nc.sync.dma_start(
                        out=out_d[b0 : b0 + NB, :, :].rearrange(
                            "b d o -> (b d) o"
                        ),
                        in_=outT[:, :],
                    )
    nc.compile()
    if reps == 1:
        _NC_CACHE = nc
    return nc


def _prep_inputs(infeature, base, W, b):
    """Host-side reshape/pad; returns per-core input maps."""
    infeature = np.asarray(infeature, dtype=np.float32)
    # [B, I, D] -> per-core [I, BLOC*D] contiguous for full-rate DMA
    infT = np.ascontiguousarray(
        infeature.reshape(NCORES, BLOC, INDIM, D).transpose(0, 2, 1, 3)
    ).reshape(NCORES, INDIM, BLOC * D)
    base = np.asarray(base, dtype=np.float32)
    W = np.asarray(W, dtype=np.float32)
    b = np.asarray(b, dtype=np.float32)

    # W'[i, o*39+j] = W[o, i*39+j]
    Wr = W.reshape(OUTDIM, INDIM, BASEDIM)
    Wp = np.ascontiguousarray(
        Wr.transpose(1, 0, 2).reshape(INDIM, F)
    )

    # base2: [B, D, J] (d-major) for single-DMA chunk loads
    base2 = np.ascontiguousarray(base.transpose(0, 2, 1))

    in_maps = []
    for c in range(NCORES):
        s = slice(c * BLOC, (c + 1) * BLOC)
        in_maps.append(
            {
                "inf": infT[c],
                "basep": base2[s],
                "w": Wp,
            }
        )
    return in_maps


def kernel(infeature, base, W, b):
    from concourse.bass_utils import run_bass_kernel_spmd

    nc = build_nc()
    in_maps = _prep_inputs(infeature, base, W, b)
    res = run_bass_kernel_spmd(nc, in_maps, core_ids=list(range(NCORES)))
    # gather cumsum samples [B, D, O]; finish with per-o diff + bias on host
    cum = np.concatenate([res.results[c]["out"] for c in range(NCORES)], axis=0)
    cum = cum.transpose(0, 2, 1)  # [B, O, D]
    out = np.empty_like(cum)
    out[:, 0, :] = cum[:, 0, :]
    np.subtract(cum[:, 1:, :], cum[:, :-1, :], out=out[:, 1:, :])
    out += np.asarray(b, np.float32).reshape(1, OUTDIM, 1)
    return np.ascontiguousarray(out).astype(np.float32)


# revision 16
# speedup vs baseline: 20.1686x; 1.0076x over previous
"""Trainium2 Bass kernel for nn_CINComp_18777597018207.

Math: out[b,o,d] = sum_{i,j} W[o, i*39+j] * infeature[b,i,d] * base[b,j,d] + bias[o]

Dataflow (per core, data-parallel over batch, 128 batch elems/core):
  - Reassociate:  out[o,n] = sum_j base[j,n] * Y[(o,j), n],
                  Y[(o,j), n] = sum_i W'[i,(o,j)] * inf[i,n],   n = (b,d)
  - Stage A (PE): Y^T[n, (o,j)] via matmuls, contraction over i (K=200, two
    k-tiles 128+72), float32r at 512-wide moving chunks (full rate), PSUM.
  - Stage B (DVE): one fused custom op per 2048-elem PSUM segment:
    running cumulative sum of Y^T * base_tiled along the (o,j) stream
    (j-inner, padded to 40). Segments chained via per-partition init scalar.
  - Stage C (DVE): per-o sums = differences of the cumsum sampled at
    j==39 positions; add bias; DMA out.
  - ACT builds the repeated base pattern (j mod 40) once per n-chunk.

Self-contained: hardcodes shapes; registers a custom DVE op at import.
"""

import numpy as np

# ---- problem constants (hardcoded per contract) ----
B, INDIM, BASEDIM, D, OUTDIM = 1024, 200, 39, 32, 200
JP = BASEDIM                 # 39, no padding needed (btile handles any period)
F = OUTDIM * JP              # 7800 (o,j) stream length
NCORES = 8
BLOC = B // NCORES           # 128 batch elems per core
NB = 4                       # batch elems per n-chunk
NCHUNK = NB * D              # 128 partitions per n-chunk
NCHUNKS = BLOC // NB         # 32
NSEG = 4                     # PSUM segments per n-chunk
SEGW = 2048                  # full segment width (4 PSUM banks)
CW = 512                     # matmul moving-dim chunk
NCW = 4                      # chunks per segment
SEG_W = [2048, 2048, 2048, 1656]   # seg 3 ends at F=7800
SEG_O = [0, 2048, 4096, 6144]
SEG_CW = [[512] * 4, [512] * 4, [512] * 4, [512, 512, 512, 120]]
TILE_REPS = 55               # 55*39 = 2145 >= 38 (max joff) + 2048

_CUSTOM_OP = None
_NC_CACHE = None


def _get_custom_op():
    """Register TT_MAC_CUMSUM_ANT: out = s0 + cumsum(in0 * in1) along free."""
    global _CUSTOM_OP
    if _CUSTOM_OP is not None:
        return _CUSTOM_OP
    import concourse.dve_ops as dve_ops_mod
    from concourse.dve_ops import DveOp, OPS
    from concourse.dve_spec import Spec, Src0, Src1, C0, AluOp, scan, lower
    from concourse.dve_uop import DveOpSpec

    name = "TT_MAC_CUMSUM_ANT"

    def ref(in0, in1, c0, c1, c2):
        a = np.asarray(in0, np.float32)
        bb = np.broadcast_to(np.asarray(in1, np.float32), a.shape)
        prod = (a * bb).reshape(a.shape[0], -1)
        cs = np.cumsum(prod, axis=1, dtype=np.float32)
        if isinstance(c0, np.ndarray):
            cs = cs + c0.reshape(-1, 1).astype(np.float32)
        else:
            cs = cs + np.float32(c0)
        return cs.reshape(a.shape)

    spec = Spec(body=scan(AluOp.ADD, Src0 * Src1, init=C0), reference=ref)
    shas = {}
    for ver in ("v3", "v4"):
        shas[ver] = DveOpSpec(
            name=name, opcode=0, uops=lower(spec, ver=ver), rd1_en=True
        ).sha(ver)
    op = DveOp(name, spec, subdim=False, uops_sha=shas)
    if name not in dve_ops_mod._SUB_OPCODE_FOR_NAME:
        OPS.append(op)
        dve_ops_mod.CUSTOM_DVE_SPECS[name] = spec
        dve_ops_mod._SUB_OPCODE_FOR_NAME[name] = (
            dve_ops_mod._CUSTOM_DVE_ROW_BASE + len(OPS) - 1
        )
        assert dve_ops_mod._SUB_OPCODE_FOR_NAME[name] < 0x20
    _CUSTOM_OP = op
    return op


def build_nc(reps=1):
    """Build (once) the per-core Bass program. SPMD: same program, 8 cores.

    reps>1 wraps the compute body in a repeat loop (benchmark builds only).
    """
    global _NC_CACHE
    if _NC_CACHE is not None and reps == 1:
        return _NC_CACHE
    import concourse.bacc as bacc
    import concourse.mybir as mybir
    from concourse.tile import TileContext

    op = _get_custom_op()
    f32 = mybir.dt.float32
    f32r = mybir.dt.float32r

    nc = bacc.Bacc("TRN2", debug=False, num_devices=NCORES)
    # inf: [BLOC, INDIM, D] fp32 bits; base2: [BLOC, D, JP]; w: [INDIM, FPAD]
    inf_d = nc.dram_tensor("inf", [INDIM, BLOC * D], f32r, kind="ExternalInput")
    base_d = nc.dram_tensor("basep", [BLOC, D, JP], f32, kind="ExternalInput")
    w_d = nc.dram_tensor("w", [INDIM, F], f32r, kind="ExternalInput")
    # out in [BLOC, D, OUTDIM] layout; host transposes to [BLOC, OUTDIM, D]
    out_d = nc.dram_tensor("out", [BLOC, D, OUTDIM], f32, kind="ExternalOutput")

    with TileContext(nc) as tc:
        with (
            tc.tile_pool(name="wpool", bufs=1) as wpool,
            tc.tile_pool(name="ipool", bufs=1) as ipool,
            tc.tile_pool(name="bse", bufs=3) as bsep,
            tc.tile_pool(name="btl", bufs=2) as btlp,
            tc.tile_pool(name="tmp", bufs=2) as tmpp,
            tc.tile_pool(name="outp", bufs=3) as outp,
            tc.tile_pool(name="ps", bufs=2, space="PSUM") as psp,
        ):
            # whole-core infeature, resident: [i, n] n=(b,d), two k-tiles,
            # quarter-split so the first matmuls wait on 1/4 of the data
            NQ = 4
            QW = BLOC * D // NQ
            inf0q, inf1q = [], []
            for q in range(NQ):
                i0 = ipool.tile([128, QW], f32r, tag=f"inf0{q}")
                nc.sync.dma_start(
                    out=i0[:, :], in_=inf_d[0:128, q * QW : (q + 1) * QW]
                )
                inf0q.append(i0)
                i1 = ipool.tile([72, QW], f32r, tag=f"inf1{q}")
                nc.sync.dma_start(
                    out=i1[:, :], in_=inf_d[128:INDIM, q * QW : (q + 1) * QW]
                )
                inf1q.append(i1)
            # persistent weights: W'[i, (o,j)] split into two k-tiles,
            # four segment tiles each (first matmul starts after seg 0 lands)
            w0s, w1s = [], []
            for seg in range(NSEG):
                fo, sw = SEG_O[seg], SEG_W[seg]
                w0 = wpool.tile([128, sw], f32r, tag=f"w0{seg}")
                nc.sync.dma_start(out=w0[:, :], in_=w_d[0:128, fo : fo + sw])
                w0s.append(w0)
                w1 = wpool.tile([72, sw], f32r, tag=f"w1{seg}")
                nc.sync.dma_start(out=w1[:, :], in_=w_d[128:INDIM, fo : fo + sw])
                w1s.append(w1)

            import contextlib

            if reps > 1:
                rep_ctx = tc.For_i(
                    0,
                    reps,
                    1,
                    hint_engines=(
                        mybir.EngineType.PE,
                        mybir.EngineType.DVE,
                        mybir.EngineType.SP,
                    ),
                )
            else:
                rep_ctx = contextlib.nullcontext()
            with rep_ctx:
                for t in range(NCHUNKS):
                    b0 = t * NB
                    n0 = t * NCHUNK
                    q, off = divmod(n0, QW)
                    lhs0f = inf0q[q][:, off : off + NCHUNK]
                    lhs1f = inf1q[q][:, off : off + NCHUNK]

                    # base chunk: [n, j] with n=(b,d) on partitions
                    bch = bsep.tile([128, JP], f32, tag="bch")
                    nc.sync.dma_start(
                        out=bch[:, :],
                        in_=base_d[b0 : b0 + NB, :, :].rearrange(
                            "b d j -> (b d) j"
                        ),
                    )
                    # repeated base pattern along the (o,j) stream (ACT)
                    btile = btlp.tile([128, TILE_REPS * JP], f32, tag="btile")
                    nc.scalar.copy(
                        out=btile[:, :].rearrange("p (r j) -> p r j", j=JP),
                        in_=bch[:, :]
                        .unsqueeze(1)
                        .broadcast_to([128, TILE_REPS, JP]),
                    )

                    tmp = tmpp.tile([128, F], f32, tag="tmp")
                    for seg in range(NSEG):
                        fo, sw = SEG_O[seg], SEG_W[seg]
                        ypsum = psp.tile([128, NCW, CW], f32, tag="ypsum")
                        for ki, (lhsf, wt) in enumerate(
                            ((lhs0f, w0s[seg]), (lhs1f, w1s[seg]))
                        ):
                            co = 0
                            for cw in SEG_CW[seg]:
                                nc.tensor.matmul(
                                    ypsum[:, co // CW, 0:cw],
                                    lhsT=lhsf,
                                    rhs=wt[:, co : co + cw],
                                    start=(ki == 0),
                                    stop=(ki == 1),
                                )
                                co += cw
                        joff = fo % JP
                        init = 0.0 if seg == 0 else tmp[:, fo - 1 : fo]
                        nc.vector._custom_dve(
                            op,
                            out=tmp[:, fo : fo + sw],
                            in0=ypsum[:, :, :].rearrange("p a b -> p (a b)")[
                                :, 0:sw
                            ],
                            in1=btile[:, joff : joff + sw],
                            s0=init,
                        )

                    # compact the cumsum samples at j==JP-1 (one DVE copy),
                    # DMA out; host computes the per-o differences + bias.
                    last = tmp[:, :].rearrange("p (o j) -> p o j", j=JP)[
                        :, :, JP - 1
                    ]
                    outT = outp.tile([128, OUTDIM], f32, tag="outT")
                    nc.scalar.copy(outT[:, :], last)
                    nc.sync.dma_start(
                        out=out_d[b0 : b0 + NB, :, :].rearrange(
                            "b d o -> (b d) o"
                        ),
                        in_=outT[:, :],
                    )
    nc.compile()
    if reps == 1:
        _NC_CACHE = nc
    return nc


def _prep_inputs(infeature, base, W, b):
    """Host-side reshape/pad; returns per-core input maps."""
    infeature = np.asarray(infeature, dtype=np.float32)
    # [B, I, D] -> per-core [I, BLOC*D] contiguous for full-rate DMA
    infT = np.ascontiguousarray(
        infeature.reshape(NCORES, BLOC, INDIM, D).transpose(0, 2, 1, 3)
    ).reshape(NCORES, INDIM, BLOC * D)
    base = np.asarray(base, dtype=np.float32)
    W = np.asarray(W, dtype=np.float32)
    b = np.asarray(b, dtype=np.float32)

    # W'[i, o*39+j] = W[o, i*39+j]
    Wr = W.reshape(OUTDIM, INDIM, BASEDIM)
    Wp = np.ascontiguousarray(
        Wr.transpose(1, 0, 2).reshape(INDIM, F)
    )

    # base2: [B, D, J] (d-major) for single-DMA chunk loads
    base2 = np.ascontiguousarray(base.transpose(0, 2, 1))

    in_maps = []
    for c in range(NCORES):
        s = slice(c * BLOC, (c + 1) * BLOC)
        in_maps.append(
            {
                "inf": infT[c],
                "basep": base2[s],
                "w": Wp,
            }
        )
    return in_maps


def kernel(infeature, base, W, b):
    from concourse.bass_utils import run_bass_kernel_spmd

    nc = build_nc()
    in_maps = _prep_inputs(infeature, base, W, b)
    res = run_bass_kernel_spmd(nc, in_maps, core_ids=list(range(NCORES)))
    # gather cumsum samples [B, D, O]; finish with per-o diff + bias on host
    cum = np.concatenate([res.results[c]["out"] for c in range(NCORES)], axis=0)
    cum = cum.transpose(0, 2, 1)  # [B, O, D]
    out = np.empty_like(cum)
    out[:, 0, :] = cum[:, 0, :]
    np.subtract(cum[:, 1:, :], cum[:, :-1, :], out=out[:, 1:, :])
    out += np.asarray(b, np.float32).reshape(1, OUTDIM, 1)
    return np.ascontiguousarray(out).astype(np.float32)


# revision 17
# speedup vs baseline: 20.5249x; 1.0177x over previous
"""Trainium2 Bass kernel for nn_CINComp_18777597018207.

Math: out[b,o,d] = sum_{i,j} W[o, i*39+j] * infeature[b,i,d] * base[b,j,d] + bias[o]

Dataflow (per core, data-parallel over batch, 128 batch elems/core):
  - Reassociate:  out[o,n] = sum_j base[j,n] * Y[(o,j), n],
                  Y[(o,j), n] = sum_i W'[i,(o,j)] * inf[i,n],   n = (b,d)
  - Stage A (PE): Y^T[n, (o,j)] via matmuls, contraction over i (K=200, two
    k-tiles 128+72), float32r at 512-wide moving chunks (full rate), PSUM.
  - Stage B (DVE): one fused custom op per 2048-elem PSUM segment:
    running cumulative sum of Y^T * base_tiled along the (o,j) stream
    (j-inner, padded to 40). Segments chained via per-partition init scalar.
  - Stage C (DVE): per-o sums = differences of the cumsum sampled at
    j==39 positions; add bias; DMA out.
  - ACT builds the repeated base pattern (j mod 40) once per n-chunk.

Self-contained: hardcodes shapes; registers a custom DVE op at import.
"""

import numpy as np

# ---- problem constants (hardcoded per contract) ----
B, INDIM, BASEDIM, D, OUTDIM = 1024, 200, 39, 32, 200
JP = BASEDIM                 # 39, no padding needed (btile handles any period)
F = OUTDIM * JP              # 7800 (o,j) stream length
NCORES = 8
BLOC = B // NCORES           # 128 batch elems per core
NB = 4                       # batch elems per n-chunk
NCHUNK = NB * D              # 128 partitions per n-chunk
NCHUNKS = BLOC // NB         # 32
NSEG = 4                     # PSUM segments per n-chunk
SEGW = 2048                  # full segment width (4 PSUM banks)
CW = 512                     # matmul moving-dim chunk
NCW = 4                      # chunks per segment
SEG_W = [2048, 2048, 2048, 1656]   # seg 3 ends at F=7800
SEG_O = [0, 2048, 4096, 6144]
SEG_CW = [[512] * 4, [512] * 4, [512] * 4, [512, 512, 512, 120]]
TILE_REPS = 55               # 55*39 = 2145 >= 38 (max joff) + 2048

_CUSTOM_OP = None
_NC_CACHE = None


def _get_custom_op():
    """Register TT_MAC_CUMSUM_ANT: out = s0 + cumsum(in0 * in1) along free."""
    global _CUSTOM_OP
    if _CUSTOM_OP is not None:
        return _CUSTOM_OP
    import concourse.dve_ops as dve_ops_mod
    from concourse.dve_ops import DveOp, OPS
    from concourse.dve_spec import Spec, Src0, Src1, C0, AluOp, scan, lower
    from concourse.dve_uop import DveOpSpec

    name = "TT_MAC_CUMSUM_ANT"

    def ref(in0, in1, c0, c1, c2):
        a = np.asarray(in0, np.float32)
        bb = np.broadcast_to(np.asarray(in1, np.float32), a.shape)
        prod = (a * bb).reshape(a.shape[0], -1)
        cs = np.cumsum(prod, axis=1, dtype=np.float32)
        if isinstance(c0, np.ndarray):
            cs = cs + c0.reshape(-1, 1).astype(np.float32)
        else:
            cs = cs + np.float32(c0)
        return cs.reshape(a.shape)

    spec = Spec(body=scan(AluOp.ADD, Src0 * Src1, init=C0), reference=ref)
    shas = {}
    for ver in ("v3", "v4"):
        shas[ver] = DveOpSpec(
            name=name, opcode=0, uops=lower(spec, ver=ver), rd1_en=True
        ).sha(ver)
    op = DveOp(name, spec, subdim=False, uops_sha=shas)
    if name not in dve_ops_mod._SUB_OPCODE_FOR_NAME:
        OPS.append(op)
        dve_ops_mod.CUSTOM_DVE_SPECS[name] = spec
        dve_ops_mod._SUB_OPCODE_FOR_NAME[name] = (
            dve_ops_mod._CUSTOM_DVE_ROW_BASE + len(OPS) - 1
        )
        assert dve_ops_mod._SUB_OPCODE_FOR_NAME[name] < 0x20
    _CUSTOM_OP = op
    return op


def build_nc(reps=1):
    """Build (once) the per-core Bass program. SPMD: same program, 8 cores.

    reps>1 wraps the compute body in a repeat loop (benchmark builds only).
    """
    global _NC_CACHE
    if _NC_CACHE is not None and reps == 1:
        return _NC_CACHE
    import concourse.bacc as bacc
    import concourse.mybir as mybir
    from concourse.tile import TileContext

    op = _get_custom_op()
    f32 = mybir.dt.float32
    f32r = mybir.dt.float32r

    nc = bacc.Bacc("TRN2", debug=False, num_devices=NCORES)
    # inf: [BLOC, INDIM, D] fp32 bits; base2: [BLOC, D, JP]; w: [INDIM, FPAD]
    inf_d = nc.dram_tensor("inf", [INDIM, BLOC * D], f32r, kind="ExternalInput")
    base_d = nc.dram_tensor("basep", [BLOC, D, JP], f32, kind="ExternalInput")
    w_d = nc.dram_tensor("w", [INDIM, F], f32r, kind="ExternalInput")
    # out in [BLOC, D, OUTDIM] layout; host transposes to [BLOC, OUTDIM, D]
    out_d = nc.dram_tensor("out", [BLOC, D, OUTDIM + 3], f32, kind="ExternalOutput")

    with TileContext(nc) as tc:
        with (
            tc.tile_pool(name="wpool", bufs=1) as wpool,
            tc.tile_pool(name="ipool", bufs=1) as ipool,
            tc.tile_pool(name="bse", bufs=3) as bsep,
            tc.tile_pool(name="btl", bufs=2) as btlp,
            tc.tile_pool(name="tmp", bufs=2) as tmpp,
            tc.tile_pool(name="outp", bufs=3) as outp,
            tc.tile_pool(name="ps", bufs=2, space="PSUM") as psp,
        ):
            # whole-core infeature, resident: [i, n] n=(b,d), two k-tiles,
            # quarter-split so the first matmuls wait on 1/4 of the data
            NQ = 4
            QW = BLOC * D // NQ
            inf0q, inf1q = [], []
            for q in range(NQ):
                i0 = ipool.tile([128, QW], f32r, tag=f"inf0{q}")
                nc.sync.dma_start(
                    out=i0[:, :], in_=inf_d[0:128, q * QW : (q + 1) * QW]
                )
                inf0q.append(i0)
                i1 = ipool.tile([72, QW], f32r, tag=f"inf1{q}")
                nc.sync.dma_start(
                    out=i1[:, :], in_=inf_d[128:INDIM, q * QW : (q + 1) * QW]
                )
                inf1q.append(i1)
            # persistent weights: W'[i, (o,j)] split into two k-tiles,
            # four segment tiles each (first matmul starts after seg 0 lands)
            w0s, w1s = [], []
            for seg in range(NSEG):
                fo, sw = SEG_O[seg], SEG_W[seg]
                w0 = wpool.tile([128, sw], f32r, tag=f"w0{seg}")
                nc.sync.dma_start(out=w0[:, :], in_=w_d[0:128, fo : fo + sw])
                w0s.append(w0)
                w1 = wpool.tile([72, sw], f32r, tag=f"w1{seg}")
                nc.sync.dma_start(out=w1[:, :], in_=w_d[128:INDIM, fo : fo + sw])
                w1s.append(w1)

            import contextlib

            if reps > 1:
                rep_ctx = tc.For_i(
                    0,
                    reps,
                    1,
                    hint_engines=(
                        mybir.EngineType.PE,
                        mybir.EngineType.DVE,
                        mybir.EngineType.SP,
                    ),
                )
            else:
                rep_ctx = contextlib.nullcontext()
            with rep_ctx:
                for t in range(NCHUNKS):
                    b0 = t * NB
                    n0 = t * NCHUNK
                    q, off = divmod(n0, QW)
                    lhs0f = inf0q[q][:, off : off + NCHUNK]
                    lhs1f = inf1q[q][:, off : off + NCHUNK]

                    # base chunk: [n, j] with n=(b,d) on partitions
                    bch = bsep.tile([128, JP], f32, tag="bch")
                    nc.sync.dma_start(
                        out=bch[:, :],
                        in_=base_d[b0 : b0 + NB, :, :].rearrange(
                            "b d j -> (b d) j"
                        ),
                    )
                    # repeated base pattern along the (o,j) stream (ACT)
                    btile = btlp.tile([128, TILE_REPS * JP], f32, tag="btile")
                    nc.scalar.copy(
                        out=btile[:, :].rearrange("p (r j) -> p r j", j=JP),
                        in_=bch[:, :]
                        .unsqueeze(1)
                        .broadcast_to([128, TILE_REPS, JP]),
                    )

                    tmp = tmpp.tile([128, F], f32, tag="tmp")
                    for seg in range(NSEG):
                        fo, sw = SEG_O[seg], SEG_W[seg]
                        ypsum = psp.tile([128, NCW, CW], f32, tag="ypsum")
                        for ki, (lhsf, wt) in enumerate(
                            ((lhs0f, w0s[seg]), (lhs1f, w1s[seg]))
                        ):
                            co = 0
                            for cw in SEG_CW[seg]:
                                nc.tensor.matmul(
                                    ypsum[:, co // CW, 0:cw],
                                    lhsT=lhsf,
                                    rhs=wt[:, co : co + cw],
                                    start=(ki == 0),
                                    stop=(ki == 1),
                                )
                                co += cw
                        joff = fo % JP
                        nc.vector._custom_dve(
                            op,
                            out=tmp[:, fo : fo + sw],
                            in0=ypsum[:, :, :].rearrange("p a b -> p (a b)")[
                                :, 0:sw
                            ],
                            in1=btile[:, joff : joff + sw],
                            s0=0.0,
                        )

                    # compact the cumsum samples at j==JP-1 (one DVE copy),
                    # DMA out; host computes the per-o differences + bias.
                    last = tmp[:, :].rearrange("p (o j) -> p o j", j=JP)[
                        :, :, JP - 1
                    ]
                    outT = outp.tile([128, OUTDIM + 3], f32, tag="outT")
                    nc.scalar.copy(outT[:, 0:OUTDIM], last)
                    for k in range(3):
                        e = SEG_O[k + 1] - 1
                        nc.scalar.copy(
                            outT[:, OUTDIM + k : OUTDIM + k + 1],
                            tmp[:, e : e + 1],
                        )
                    nc.sync.dma_start(
                        out=out_d[b0 : b0 + NB, :, :].rearrange(
                            "b d o -> (b d) o"
                        ),
                        in_=outT[:, :],
                    )
    nc.compile()
    if reps == 1:
        _NC_CACHE = nc
    return nc


def _prep_inputs(infeature, base, W, b):
    """Host-side reshape/pad; returns per-core input maps."""
    infeature = np.asarray(infeature, dtype=np.float32)
    # [B, I, D] -> per-core [I, BLOC*D] contiguous for full-rate DMA
    infT = np.ascontiguousarray(
        infeature.reshape(NCORES, BLOC, INDIM, D).transpose(0, 2, 1, 3)
    ).reshape(NCORES, INDIM, BLOC * D)
    base = np.asarray(base, dtype=np.float32)
    W = np.asarray(W, dtype=np.float32)
    b = np.asarray(b, dtype=np.float32)

    # W'[i, o*39+j] = W[o, i*39+j]
    Wr = W.reshape(OUTDIM, INDIM, BASEDIM)
    Wp = np.ascontiguousarray(
        Wr.transpose(1, 0, 2).reshape(INDIM, F)
    )

    # base2: [B, D, J] (d-major) for single-DMA chunk loads
    base2 = np.ascontiguousarray(base.transpose(0, 2, 1))

    in_maps = []
    for c in range(NCORES):
        s = slice(c * BLOC, (c + 1) * BLOC)
        in_maps.append(
            {
                "inf": infT[c],
                "basep": base2[s],
                "w": Wp,
            }
        )
    return in_maps


def kernel(infeature, base, W, b):
    from concourse.bass_utils import run_bass_kernel_spmd

    nc = build_nc()
    in_maps = _prep_inputs(infeature, base, W, b)
    res = run_bass_kernel_spmd(nc, in_maps, core_ids=list(range(NCORES)))
    # gather per-segment cumsum samples [B, D, O+3]; finish on host:
    # per-o diffs, stitch the 3 segment boundaries, add bias.
    cum = np.concatenate([res.results[c]["out"] for c in range(NCORES)], axis=0)
    samp = cum[:, :, :OUTDIM]          # [B, D, O] local cumsums at j==38
    ends = cum[:, :, OUTDIM:]          # [B, D, 3] cumsum at segment ends
    out = np.empty_like(samp)
    out[:, :, 0] = samp[:, :, 0]
    np.subtract(samp[:, :, 1:], samp[:, :, :-1], out=out[:, :, 1:])
    # o-groups straddling segment boundaries (positions 2048, 4096, 6144):
    # out[o] = (seg_end - samp[o-1]) + samp[o]
    for of, e in ((52, 0), (105, 1), (157, 2)):
        out[:, :, of] = (ends[:, :, e] - samp[:, :, of - 1]) + samp[:, :, of]
    out = out.transpose(0, 2, 1)       # [B, O, D]
    out = out + np.asarray(b, np.float32).reshape(1, OUTDIM, 1)
    return np.ascontiguousarray(out).astype(np.float32)


# revision 18
# speedup vs baseline: 20.6535x; 1.0063x over previous
"""Trainium2 Bass kernel for nn_CINComp_18777597018207.

Math: out[b,o,d] = sum_{i,j} W[o, i*39+j] * infeature[b,i,d] * base[b,j,d] + bias[o]

Dataflow (per core, data-parallel over batch, 128 batch elems/core):
  - Reassociate:  out[o,n] = sum_j base[j,n] * Y[(o,j), n],
                  Y[(o,j), n] = sum_i W'[i,(o,j)] * inf[i,n],   n = (b,d)
  - Stage A (PE): Y^T[n, (o,j)] via matmuls, contraction over i (K=200, two
    k-tiles 128+72), float32r at >=256-wide moving chunks (full rate), PSUM.
  - Stage B (DVE): one fused custom op per 4-bank PSUM segment: local
    cumulative sum of Y^T * base_tiled along the (o,j) stream (j-inner,
    period 39). Segments are independent (no init chaining) so consecutive
    ops overlap their fixed PSUM-access windows; 4+4-bank double buffering.
  - Stage C (ACT): compact the cumsum samples at j==38 plus the 3
    segment-end values; DMA out [n, 203].
  - Host: per-o differences, segment-boundary stitching (o in {52,105,157}),
    bias add, final [B,O,D] transpose.
  - ACT also builds the repeated base pattern (j mod 39) once per n-chunk.

Self-contained: hardcodes shapes; registers a custom DVE op at import.
"""

import numpy as np

# ---- problem constants (hardcoded per contract) ----
B, INDIM, BASEDIM, D, OUTDIM = 1024, 200, 39, 32, 200
JP = BASEDIM                 # 39, no padding needed (btile handles any period)
F = OUTDIM * JP              # 7800 (o,j) stream length
NCORES = 8
BLOC = B // NCORES           # 128 batch elems per core
NB = 4                       # batch elems per n-chunk
NCHUNK = NB * D              # 128 partitions per n-chunk
NCHUNKS = BLOC // NB         # 32
NSEG = 4                     # PSUM segments per n-chunk
SEGW = 2048                  # full segment width (4 PSUM banks)
CW = 512                     # matmul moving-dim chunk
NCW = 4                      # chunks per segment
SEG_W = [2048, 2048, 2048, 1656]   # seg 3 ends at F=7800
SEG_O = [0, 2048, 4096, 6144]
SEG_CW = [[512] * 4, [512] * 4, [512] * 4, [512, 512, 512, 120]]
TILE_REPS = 55               # 55*39 = 2145 >= 38 (max joff) + 2048

_CUSTOM_OP = None
_NC_CACHE = None


def _get_custom_op():
    """Register TT_MAC_CUMSUM_ANT: out = s0 + cumsum(in0 * in1) along free."""
    global _CUSTOM_OP
    if _CUSTOM_OP is not None:
        return _CUSTOM_OP
    import concourse.dve_ops as dve_ops_mod
    from concourse.dve_ops import DveOp, OPS
    from concourse.dve_spec import Spec, Src0, Src1, C0, AluOp, scan, lower
    from concourse.dve_uop import DveOpSpec

    name = "TT_MAC_CUMSUM_ANT"

    def ref(in0, in1, c0, c1, c2):
        a = np.asarray(in0, np.float32)
        bb = np.broadcast_to(np.asarray(in1, np.float32), a.shape)
        prod = (a * bb).reshape(a.shape[0], -1)
        cs = np.cumsum(prod, axis=1, dtype=np.float32)
        if isinstance(c0, np.ndarray):
            cs = cs + c0.reshape(-1, 1).astype(np.float32)
        else:
            cs = cs + np.float32(c0)
        return cs.reshape(a.shape)

    spec = Spec(body=scan(AluOp.ADD, Src0 * Src1, init=C0), reference=ref)
    shas = {}
    for ver in ("v3", "v4"):
        shas[ver] = DveOpSpec(
            name=name, opcode=0, uops=lower(spec, ver=ver), rd1_en=True
        ).sha(ver)
    op = DveOp(name, spec, subdim=False, uops_sha=shas)
    if name not in dve_ops_mod._SUB_OPCODE_FOR_NAME:
        OPS.append(op)
        dve_ops_mod.CUSTOM_DVE_SPECS[name] = spec
        dve_ops_mod._SUB_OPCODE_FOR_NAME[name] = (
            dve_ops_mod._CUSTOM_DVE_ROW_BASE + len(OPS) - 1
        )
        assert dve_ops_mod._SUB_OPCODE_FOR_NAME[name] < 0x20
    _CUSTOM_OP = op
    return op


def build_nc(reps=1):
    """Build (once) the per-core Bass program. SPMD: same program, 8 cores.

    reps>1 wraps the compute body in a repeat loop (benchmark builds only).
    """
    global _NC_CACHE
    if _NC_CACHE is not None and reps == 1:
        return _NC_CACHE
    import concourse.bacc as bacc
    import concourse.mybir as mybir
    from concourse.tile import TileContext

    op = _get_custom_op()
    f32 = mybir.dt.float32
    f32r = mybir.dt.float32r

    nc = bacc.Bacc("TRN2", debug=False, num_devices=NCORES)
    # inf: [BLOC, INDIM, D] fp32 bits; base2: [BLOC, D, JP]; w: [INDIM, FPAD]
    inf_d = nc.dram_tensor("inf", [INDIM, BLOC * D], f32r, kind="ExternalInput")
    base_d = nc.dram_tensor("basep", [BLOC, D, JP], f32, kind="ExternalInput")
    w_d = nc.dram_tensor("w", [INDIM, F], f32r, kind="ExternalInput")
    # out in [BLOC, D, OUTDIM] layout; host transposes to [BLOC, OUTDIM, D]
    out_d = nc.dram_tensor("out", [BLOC, D, OUTDIM + 3], f32, kind="ExternalOutput")

    with TileContext(nc) as tc:
        with (
            tc.tile_pool(name="wpool", bufs=1) as wpool,
            tc.tile_pool(name="ipool", bufs=1) as ipool,
            tc.tile_pool(name="bse", bufs=3) as bsep,
            tc.tile_pool(name="btl", bufs=2) as btlp,
            tc.tile_pool(name="tmp", bufs=2) as tmpp,
            tc.tile_pool(name="outp", bufs=3) as outp,
            tc.tile_pool(name="ps", bufs=2, space="PSUM") as psp,
        ):
            # whole-core infeature, resident: [i, n] n=(b,d), two k-tiles,
            # quarter-split so the first matmuls wait on 1/4 of the data
            NQ = 4
            QW = BLOC * D // NQ
            inf0q, inf1q = [], []
            for q in range(NQ):
                i0 = ipool.tile([128, QW], f32r, tag=f"inf0{q}")
                nc.sync.dma_start(
                    out=i0[:, :], in_=inf_d[0:128, q * QW : (q + 1) * QW]
                )
                inf0q.append(i0)
                i1 = ipool.tile([72, QW], f32r, tag=f"inf1{q}")
                nc.sync.dma_start(
                    out=i1[:, :], in_=inf_d[128:INDIM, q * QW : (q + 1) * QW]
                )
                inf1q.append(i1)
            # persistent weights: W'[i, (o,j)] split into two k-tiles,
            # four segment tiles each (first matmul starts after seg 0 lands)
            w0s, w1s = [], []
            for seg in range(NSEG):
                fo, sw = SEG_O[seg], SEG_W[seg]
                w0 = wpool.tile([128, sw], f32r, tag=f"w0{seg}")
                nc.sync.dma_start(out=w0[:, :], in_=w_d[0:128, fo : fo + sw])
                w0s.append(w0)
                w1 = wpool.tile([72, sw], f32r, tag=f"w1{seg}")
                nc.sync.dma_start(out=w1[:, :], in_=w_d[128:INDIM, fo : fo + sw])
                w1s.append(w1)

            import contextlib

            if reps > 1:
                rep_ctx = tc.For_i(
                    0,
                    reps,
                    1,
                    hint_engines=(
                        mybir.EngineType.PE,
                        mybir.EngineType.DVE,
                        mybir.EngineType.SP,
                    ),
                )
            else:
                rep_ctx = contextlib.nullcontext()
            with rep_ctx:
                for t in range(NCHUNKS):
                    b0 = t * NB
                    n0 = t * NCHUNK
                    q, off = divmod(n0, QW)
                    lhs0f = inf0q[q][:, off : off + NCHUNK]
                    lhs1f = inf1q[q][:, off : off + NCHUNK]

                    # base chunk: [n, j] with n=(b,d) on partitions
                    bch = bsep.tile([128, JP], f32, tag="bch")
                    nc.sync.dma_start(
                        out=bch[:, :],
                        in_=base_d[b0 : b0 + NB, :, :].rearrange(
                            "b d j -> (b d) j"
                        ),
                    )
                    # repeated base pattern along the (o,j) stream (ACT)
                    btile = btlp.tile([128, TILE_REPS * JP], f32, tag="btile")
                    nc.scalar.copy(
                        out=btile[:, :].rearrange("p (r j) -> p r j", j=JP),
                        in_=bch[:, :]
                        .unsqueeze(1)
                        .broadcast_to([128, TILE_REPS, JP]),
                    )

                    tmp = tmpp.tile([128, F], f32, tag="tmp")
                    for seg in range(NSEG):
                        fo, sw = SEG_O[seg], SEG_W[seg]
                        ypsum = psp.tile([128, NCW, CW], f32, tag="ypsum")
                        for ki, (lhsf, wt) in enumerate(
                            ((lhs0f, w0s[seg]), (lhs1f, w1s[seg]))
                        ):
                            co = 0
                            for cw in SEG_CW[seg]:
                                nc.tensor.matmul(
                                    ypsum[:, co // CW, 0:cw],
                                    lhsT=lhsf,
                                    rhs=wt[:, co : co + cw],
                                    start=(ki == 0),
                                    stop=(ki == 1),
                                )
                                co += cw
                        joff = fo % JP
                        nc.vector._custom_dve(
                            op,
                            out=tmp[:, fo : fo + sw],
                            in0=ypsum[:, :, :].rearrange("p a b -> p (a b)")[
                                :, 0:sw
                            ],
                            in1=btile[:, joff : joff + sw],
                            s0=0.0,
                        )

                    # compact the cumsum samples at j==JP-1 (one DVE copy),
                    # DMA out; host computes the per-o differences + bias.
                    last = tmp[:, :].rearrange("p (o j) -> p o j", j=JP)[
                        :, :, JP - 1
                    ]
                    outT = outp.tile([128, OUTDIM + 3], f32, tag="outT")
                    nc.scalar.copy(outT[:, 0:OUTDIM], last)
                    for k in range(3):
                        e = SEG_O[k + 1] - 1
                        nc.scalar.copy(
                            outT[:, OUTDIM + k : OUTDIM + k + 1],
                            tmp[:, e : e + 1],
                        )
                    nc.sync.dma_start(
                        out=out_d[b0 : b0 + NB, :, :].rearrange(
                            "b d o -> (b d) o"
                        ),
                        in_=outT[:, :],
                    )
    nc.compile()
    if reps == 1:
        _NC_CACHE = nc
    return nc


def _prep_inputs(infeature, base, W, b):
    """Host-side reshape/pad; returns per-core input maps."""
    infeature = np.asarray(infeature, dtype=np.float32)
    # [B, I, D] -> per-core [I, BLOC*D] contiguous for full-rate DMA
    infT = np.ascontiguousarray(
        infeature.reshape(NCORES, BLOC, INDIM, D).transpose(0, 2, 1, 3)
    ).reshape(NCORES, INDIM, BLOC * D)
    base = np.asarray(base, dtype=np.float32)
    W = np.asarray(W, dtype=np.float32)
    b = np.asarray(b, dtype=np.float32)

    # W'[i, o*39+j] = W[o, i*39+j]
    Wr = W.reshape(OUTDIM, INDIM, BASEDIM)
    Wp = np.ascontiguousarray(
        Wr.transpose(1, 0, 2).reshape(INDIM, F)
    )

    # base2: [B, D, J] (d-major) for single-DMA chunk loads
    base2 = np.ascontiguousarray(base.transpose(0, 2, 1))

    in_maps = []
    for c in range(NCORES):
        s = slice(c * BLOC, (c + 1) * BLOC)
        in_maps.append(
            {
                "inf": infT[c],
                "basep": base2[s],
                "w": Wp,
            }
        )
    return in_maps


def kernel(infeature, base, W, b):
    from concourse.bass_utils import run_bass_kernel_spmd

    nc = build_nc()
    in_maps = _prep_inputs(infeature, base, W, b)
    res = run_bass_kernel_spmd(nc, in_maps, core_ids=list(range(NCORES)))
    # gather per-segment cumsum samples [B, D, O+3]; finish on host:
    # per-o diffs, stitch the 3 segment boundaries, add bias.
    cum = np.concatenate([res.results[c]["out"] for c in range(NCORES)], axis=0)
    samp = cum[:, :, :OUTDIM]          # [B, D, O] local cumsums at j==38
    ends = cum[:, :, OUTDIM:]          # [B, D, 3] cumsum at segment ends
    out = np.empty_like(samp)
    out[:, :, 0] = samp[:, :, 0]
    np.subtract(samp[:, :, 1:], samp[:, :, :-1], out=out[:, :, 1:])
    # o-groups straddling segment boundaries (positions 2048, 4096, 6144):
    # out[o] = (seg_end - samp[o-1]) + samp[o]
    for of, e in ((52, 0), (105, 1), (157, 2)):
        out[:, :, of] = (ends[:, :, e] - samp[:, :, of - 1]) + samp[:, :, of]
    out = out.transpose(0, 2, 1)       # [B, O, D]
    out = out + np.asarray(b, np.float32).reshape(1, OUTDIM, 1)
    return np.ascontiguousarray(out).astype(np.float32)
